# revision 25
# baseline (speedup 1.0000x reference)
"""GQA causal attention (RoPE) on 8 Trainium2 NeuronCores.

Sharding (tensor-parallel over heads, per the hint):
  core c owns q-heads {2c, 2c+1} and kv-head c//2.
  Each core computes its 2 heads' attention over the full sequence and a
  partial output projection out_c.T = wo[:, 128c:128c+128] @ att_c  (shape
  [1024, 4096]); the final all-reduce over cores is the host-side unshard.

Device-side per core (v4 — fused + software-pipelined over 512-row chunks):
  loop n: proj(n) -> rope(n) -> v^T(n) -> normalize+wo(n-1) -> attention(n).
  The chunk-(n-1) normalization (DVE reciprocal chain) overlaps chunk-n PE
  work, so the PE stream never waits on it and stays HAM-warm.

  - All matmuls bf16 (1 cycle/row; fp32r streams at ~2 cycles/row), fp32 PSUM.
  - RoPE in fp32 from PSUM: host-side even/odd permutation folded into wq/wk,
    sign-folded sin tile, partition-block swap via SBUF->SBUF DMA; bf16 out.
  - scores^T blocks [sk=128, sq=512]: single K=64 matmuls; the two heads hit
    disjoint PE row groups (k duplicated in both halves) -> concurrent.
  - Causal masks accumulated on the PE (identity matmul, -1e6 bf16 additive
    masks) before a grouped PSUM exp on ScalarE.
  - AV with ones-augmented V^T so softmax denominators fall out of the same
    matmul; V^T via PE transpose-mode, evacuated by ScalarE copies.
  - Normalization: raw AV staged to SBUF (f32r), both heads' denominators
    merged into one K=2 broadcast matmul, one reciprocal per chunk,
    per-column scale, then the 8 wo matmuls for the chunk.
"""
import numpy as np
import ml_dtypes
from contextlib import ExitStack

import concourse.bacc as bacc
import concourse.tile as tile
import concourse.mybir as mybir
from concourse.bass_utils import run_bass_kernel_spmd

DIM = 1024
N_HEADS = 16
N_KV = 4
HD = 64
SEQ = 4096
NCORES = 8

SQ = 512            # query-chunk (free dim of score blocks)
SK = 128            # key-chunk (partition dim of score blocks)
NQ = SEQ // SQ      # 8
NR = DIM // 128     # 8 contraction chunks for projections
NJ = SEQ // SK      # 32 key chunks
GROUP = 2           # score blocks per exp instruction ([128, 1024] PSUM)
MASKVAL = -1.0e6

f32 = mybir.dt.float32
f32r = mybir.dt.float32r
bf16 = mybir.dt.bfloat16
FT = mybir.ActivationFunctionType

_CACHE = {}


def _emit(nc):
    xT = nc.dram_tensor("xT", [DIM, SEQ], bf16, kind="ExternalInput").ap()
    wq_l = nc.dram_tensor("wq_l", [128, DIM], bf16, kind="ExternalInput").ap()
    wkv_l = nc.dram_tensor("wkv_l", [128, DIM], bf16, kind="ExternalInput").ap()
    wo_l = nc.dram_tensor("wo_l", [128, DIM], bf16, kind="ExternalInput").ap()
    cos4_d = nc.dram_tensor("cos4", [128, SEQ], f32, kind="ExternalInput").ap()
    sin4_d = nc.dram_tensor("sin4", [128, SEQ], f32, kind="ExternalInput").ap()
    mask_d = nc.dram_tensor("mask", [128, 4 * SQ], bf16, kind="ExternalInput").ap()
    on_d = nc.dram_tensor("ones32v2", [128, NJ], bf16, kind="ExternalInput").ap()
    id_d = nc.dram_tensor("ident", [128, 128], bf16, kind="ExternalInput").ap()
    sel_d = nc.dram_tensor("sel2", [128, 128], f32r, kind="ExternalInput").ap()
    out_d = nc.dram_tensor("out", [DIM, SEQ], f32, kind="ExternalOutput").ap()

    with tile.TileContext(nc) as tc, ExitStack() as ctx:
        const = ctx.enter_context(tc.tile_pool(name="const", bufs=1))
        main = ctx.enter_context(tc.tile_pool(name="main", bufs=1))

        wq_sb = const.tile([128, DIM], bf16)
        wkv_sb = const.tile([128, DIM], bf16)
        wo_sb = const.tile([128, DIM], bf16)
        cos_sb = const.tile([128, SEQ], f32)
        sin_sb = const.tile([128, SEQ], f32)
        msk_sb = const.tile([128, 4 * SQ], bf16)
        id_sb = const.tile([128, 128], bf16)
        sel_sb = const.tile([128, 128], f32r)

        qrot = main.tile([128, SEQ], bf16)      # 2 heads d-major (rope'd)
        krot = main.tile([128, SEQ], bf16)      # k duplicated in both halves
        v_sb = main.tile([HD, SEQ], bf16)       # v d-major
        vt = main.tile([128, NJ, 128], bf16)    # v^T + ones column (aligned slots)
        attS = main.tile([128, SEQ], bf16)      # stacked normalized att (j-major)
        att1 = main.tile([HD, SEQ], bf16)       # head-1 att staging (lanes 0-63)
        au0 = main.tile([HD + 1, SEQ], f32r)    # raw AV + denom staging, head 0
        au1 = main.tile([HD + 1, SEQ], f32r)    # head 1
        d2 = main.tile([66, SEQ], f32r)         # merged denoms (rows 64, 65)


        with (
            tc.tile_pool(name="xp", bufs=8) as xp,
            tc.tile_pool(name="pp", bufs=1, space="PSUM") as pp,
            tc.tile_pool(name="rp", bufs=2) as rp,
            tc.tile_pool(name="sp", bufs=2, space="PSUM") as sp,
            tc.tile_pool(name="ap", bufs=1, space="PSUM") as ap,
            tc.tile_pool(name="ep", bufs=5) as ep,
            tc.tile_pool(name="rbp", bufs=2) as rbp,
            tc.tile_pool(name="op", bufs=4) as op,
        ):
            def endgame_prep(k):
                sk0 = k * SQ
                nc.gpsimd.dma_start(d2[64:65, sk0:sk0 + SQ], au0[HD:HD + 1, sk0:sk0 + SQ])
                nc.gpsimd.dma_start(d2[65:66, sk0:sk0 + SQ], au1[HD:HD + 1, sk0:sk0 + SQ])

            def endgame_bc(k):
                sk0 = k * SQ
                bc = pp.tile([128, SQ], f32, tag="pq", name=f"bc_{k}")
                nc.tensor.matmul(bc[:], sel_sb[64:66, :], d2[64:66, sk0:sk0 + SQ],
                                 start=True, stop=True)
                rb = rbp.tile([128, SQ], f32, tag="rb")
                nc.vector.reciprocal(rb[:], bc[:])
                rb1 = rbp.tile([HD, SQ], f32, tag="rb1")
                nc.gpsimd.dma_start(rb1[:], rb[64:64 + HD, :])
                nc.vector.tensor_mul(attS[0:HD, sk0:sk0 + SQ],
                                     au0[0:HD, sk0:sk0 + SQ].bitcast(f32), rb[0:HD, :])
                nc.vector.tensor_mul(att1[:, sk0:sk0 + SQ],
                                     au1[0:HD, sk0:sk0 + SQ].bitcast(f32), rb1[:])
                nc.gpsimd.dma_start(attS[64:128, sk0:sk0 + SQ], att1[:, sk0:sk0 + SQ])

            def endgame_wo(k):
                sk0 = k * SQ
                for m in range(8):
                    pw = pp.tile([128, SQ], f32, tag=("pkv" if m % 2 == 0 else "pq"),
                                 name=f"pw_{k}_{m}")
                    nc.tensor.matmul(pw[:], wo_sb[:, 128 * m:128 * (m + 1)],
                                     attS[:, sk0:sk0 + SQ], start=True, stop=True)
                    ot = op.tile([128, SQ], f32)
                    nc.vector.tensor_copy(ot[:], pw[:])
                    eng = nc.sync if m % 2 == 0 else nc.scalar
                    eng.dma_start(out_d[128 * m:128 * (m + 1), sk0:sk0 + SQ], ot[:])

            for n in range(NQ):
                s0 = n * SQ
                # ---- projections ----
                pq = pp.tile([128, SQ], f32, tag="pq")
                pkv = pp.tile([128, SQ], f32, tag="pkv")
                for r in range(NR):
                    if n == 0:
                        nc.sync.dma_start(wq_sb[:, 128 * r:128 * (r + 1)],
                                          wq_l[:, 128 * r:128 * (r + 1)])
                        nc.sync.dma_start(wkv_sb[:, 128 * r:128 * (r + 1)],
                                          wkv_l[:, 128 * r:128 * (r + 1)])
                    xt = xp.tile([128, SQ], bf16)
                    nc.sync.dma_start(xt[:], xT[128 * r:128 * (r + 1), s0:s0 + SQ])
                    nc.tensor.matmul(pq[:], wq_sb[:, 128 * r:128 * (r + 1)], xt[:],
                                     start=(r == 0), stop=(r == NR - 1))
                    nc.tensor.matmul(pkv[:], wkv_sb[:, 128 * r:128 * (r + 1)], xt[:],
                                     start=(r == 0), stop=(r == NR - 1))
                # trig chunks (queue behind this iteration's xt loads)
                nc.sync.dma_start(cos_sb[:, s0:s0 + SQ], cos4_d[:, s0:s0 + SQ])
                nc.sync.dma_start(sin_sb[:, s0:s0 + SQ], sin4_d[:, s0:s0 + SQ])
                if n == 0:
                    nc.sync.dma_start(msk_sb[:], mask_d[:])
                    nc.sync.dma_start(id_sb[:], id_d[:])
                    nc.sync.dma_start(vt[:, :, HD:HD + 1], on_d[:])
                    nc.sync.dma_start(wo_sb[:], wo_l[:])
                    nc.sync.dma_start(sel_sb[:], sel_d[:])
                # ---- rope q ----
                a_t = rp.tile([128, SQ], f32, tag="ta")
                c_t = rp.tile([128, SQ], f32, tag="tc")
                b_t = rp.tile([128, SQ], f32, tag="tb")
                nc.vector.tensor_mul(a_t[:], pq[:], cos_sb[:, s0:s0 + SQ])
                nc.vector.tensor_mul(c_t[:], pq[:], sin_sb[:, s0:s0 + SQ])
                nc.gpsimd.dma_start(b_t[0:32, :], c_t[32:64, :])
                nc.gpsimd.dma_start(b_t[32:64, :], c_t[0:32, :])
                nc.gpsimd.dma_start(b_t[64:96, :], c_t[96:128, :])
                nc.gpsimd.dma_start(b_t[96:128, :], c_t[64:96, :])
                nc.vector.tensor_add(qrot[:, s0:s0 + SQ], a_t[:], b_t[:])
                # ---- rope k (rows 64:128; v occupies rows 0:64) ----
                ak = rp.tile([128, SQ], f32, tag="ta")
                ck = rp.tile([128, SQ], f32, tag="tc")
                bk = rp.tile([128, SQ], f32, tag="tb")
                nc.vector.tensor_mul(ak[64:128, :], pkv[64:128, :], cos_sb[64:128, s0:s0 + SQ])
                nc.vector.tensor_mul(ck[64:128, :], pkv[64:128, :], sin_sb[64:128, s0:s0 + SQ])
                nc.gpsimd.dma_start(bk[64:96, :], ck[96:128, :])
                nc.gpsimd.dma_start(bk[96:128, :], ck[64:96, :])
                nc.vector.tensor_add(krot[64:128, s0:s0 + SQ], ak[64:128, :], bk[64:128, :])
                nc.gpsimd.dma_start(krot[0:64, s0:s0 + SQ], krot[64:128, s0:s0 + SQ])
                # ---- v -> bf16, PE transpose into vt (ScalarE evacuates) ----
                nc.vector.tensor_copy(v_sb[:, s0:s0 + SQ], pkv[0:64, :])
                for j in range(4 * n, 4 * n + 4):
                    pt = sp.tile([SK, HD], bf16, tag="sc", name=f"pt_{j}")
                    nc.tensor.transpose(pt[:], v_sb[:, SK * j:SK * (j + 1)],
                                        id_sb[0:HD, 0:HD])
                    nc.vector.tensor_copy(vt[:, j, 0:HD], pt[:])

                # ---- attention ----
                nsk = 4 * (n + 1)
                av = [ap.tile([HD + 1, SQ], f32, tag=f"av{h}", name=f"av{h}_{n}")
                      for h in (0, 1)]
                blocks = [(j, h) for j in range(nsk) for h in (0, 1)]
                groups = [blocks[g0:g0 + GROUP] for g0 in range(0, len(blocks), GROUP)]
                pend = []   # (grp, et) awaiting AV emission (lag 2)

                def flush_av(n_=n):
                    grp_, et_ = pend.pop(0)
                    nsk_ = 4 * (n_ + 1)
                    for i_, (j_, h_) in enumerate(grp_):
                        dd_ = max(0, SK * j_ - n_ * SQ)
                        nc.tensor.matmul(
                            av[h_][:, dd_:SQ], vt[:, j_, 0:HD + 1],
                            et_[:, i_ * SQ + dd_:(i_ + 1) * SQ],
                            start=(j_ == 0), stop=(j_ == nsk_ - 1),
                        )

                for gi, grp in enumerate(groups):
                    sc = sp.tile([128, GROUP * SQ], f32, tag="sc")
                    for i, (j, h) in enumerate(grp):
                        o = i * SQ
                        delta = SK * j - s0
                        diag = delta > 0
                        dd = max(0, delta)
                        nc.tensor.matmul(
                            sc[:, o + dd:o + SQ],
                            krot[64 * h:64 * h + 64, SK * j:SK * (j + 1)],
                            qrot[64 * h:64 * h + 64, s0 + dd:s0 + SQ],
                            start=True, stop=(delta < 0),
                        )
                        if delta >= 0:
                            db = (delta // SK) * SQ + dd
                            nc.tensor.matmul(sc[:, o + delta:o + delta + SK],
                                             id_sb[:], msk_sb[:, db:db + SK],
                                             start=False, stop=True)
                    ew = len(grp) * SQ
                    et = ep.tile([128, GROUP * SQ], bf16, tag="et")
                    nc.scalar.activation(et[:, 0:ew], sc[:, 0:ew], FT.Exp, scale=0.125)
                    pend.append((grp, et))
                    if len(pend) > 3:
                        flush_av()
                    if n > 0:
                        if gi == 1:
                            endgame_bc(n - 1)
                        elif gi == 3:
                            endgame_wo(n - 1)
                while pend:
                    flush_av()
                # ---- stage raw AV, free banks ----
                nc.vector.tensor_copy(au0[:, s0:s0 + SQ], av[0][:])
                nc.vector.tensor_copy(au1[:, s0:s0 + SQ], av[1][:])
                endgame_prep(n)

            endgame_bc(NQ - 1)
            endgame_wo(NQ - 1)


def _build():
    if "nc" in _CACHE:
        return _CACHE["nc"]
    nc = bacc.Bacc("TRN2", target_bir_lowering=False, debug=False, num_devices=NCORES)
    _emit(nc)
    nc.compile()
    _CACHE["nc"] = nc
    return nc


def _host_inputs(x, freqs_cos, freqs_sin, wq, wk, wv, wo):
    x = np.asarray(x, np.float32)
    freqs_cos = np.asarray(freqs_cos, np.float32)
    freqs_sin = np.asarray(freqs_sin, np.float32)
    wq = np.asarray(wq, np.float32)
    wk = np.asarray(wk, np.float32)
    wv = np.asarray(wv, np.float32)
    wo = np.asarray(wo, np.float32)

    xT = np.ascontiguousarray(x[0].T).astype(ml_dtypes.bfloat16)   # [1024, 4096]
    cosT = freqs_cos.T                                             # [32, 4096]
    sinT = freqs_sin.T
    cos4 = np.ascontiguousarray(np.tile(cosT, (4, 1)))             # [128, 4096]
    sin4 = np.ascontiguousarray(
        np.concatenate([sinT, -sinT, sinT, -sinT], axis=0))

    # diagonal-block causal masks for delta in {0,128,256,384}
    p = np.arange(SK)[:, None]
    f = np.arange(SQ)[None, :]
    mask = np.concatenate(
        [np.where(SK * d + p <= f, 0.0, MASKVAL) for d in range(4)],
        axis=1).astype(ml_dtypes.bfloat16)                         # [128, 2048]

    ones32 = np.ones((128, NJ), dtype=ml_dtypes.bfloat16)
    ident = np.eye(128, dtype=ml_dtypes.bfloat16)
    sel2 = np.zeros((128, 128), dtype=np.float32)
    sel2[64, 0:64] = 1.0
    sel2[65, 64:128] = 1.0

    perm = np.concatenate([np.arange(0, HD, 2), np.arange(1, HD, 2)])

    def fold(w):  # [128(m), 1024(d)] -> lhsT layout [128(p), 8r*128+m]
        return np.ascontiguousarray(
            w.reshape(128, NR, 128).transpose(2, 1, 0).reshape(128, DIM)
        ).astype(ml_dtypes.bfloat16)

    in_maps = []
    for c in range(NCORES):
        g = c // 2
        wq_c = wq[128 * c:128 * (c + 1)].reshape(2, HD, DIM)[:, perm, :].reshape(128, DIM)
        wk_g = wk[HD * g:HD * (g + 1)][perm]
        wv_g = wv[HD * g:HD * (g + 1)]
        wkv_c = np.concatenate([wv_g, wk_g], axis=0)        # v rows 0:64, k rows 64:128
        wo_c = np.ascontiguousarray(wo[:, 128 * c:128 * (c + 1)].T).astype(
            ml_dtypes.bfloat16)                              # [128(j), 1024(o)]
        in_maps.append({
            "xT": xT,
            "wq_l": fold(wq_c),
            "wkv_l": fold(wkv_c),
            "wo_l": wo_c,
            "cos4": cos4,
            "sin4": sin4,
            "mask": mask,
            "ones32v2": ones32,
            "ident": ident,
            "sel2": sel2,
        })
    return in_maps


def kernel(x, freqs_cos, freqs_sin, wq, wk, wv, wo, _trace=False, _trace_kwargs=None):
    nc = _build()
    in_maps = _host_inputs(x, freqs_cos, freqs_sin, wq, wk, wv, wo)
    kw = {}
    if _trace:
        kw.update(trace=True, **(_trace_kwargs or {}))
    res = run_bass_kernel_spmd(nc, in_maps, core_ids=list(range(NCORES)), **kw)
    acc = np.zeros((DIM, SEQ), np.float32)
    for c in range(NCORES):
        acc += res.results[c]["out"]
    out = np.ascontiguousarray(acc.T).reshape(1, SEQ, DIM)
    if _trace:
        kernel._last_results = res
    return out
